# revision 18
# baseline (speedup 1.0000x reference)
"""Trainium2 Bass kernel for the ChunkedSIEVE model (segment_reduce).

Math (see reference):
  x[b,v,:]  = tanh(feat[b,v,:] @ W_feat + b_feat + pos[b,v]*1e-6 * w_pos)
              + gene_table[gene_ids[b,v]]
  emb[b]    = mean_v x[b,v,:]                      (mask is all ones)
  scores[b] = tanh(emb @ W_att1 + b_att1) @ W_att2 (+ b_att2, cancels in softmax)
  per-sample (8 contiguous chunks) softmax over scores -> w
  out[s]    = sum_b w[b] * (emb[b] @ W_cls) + b_cls

Strategy: data-parallel over chunks, 256 chunks (32 samples) per core.
Everything stays in [D x chunk] layout:
  - PE computes z = [W_feat; w_pos]^T @ featT in bf16 (K=65: 64 features +
    the scaled-position row appended on the host).
  - ACT applies tanh with the per-partition b_feat bias straight out of
    PSUM, writing bf16 tiles.
  - The V-sum of the tanh term runs as a pairwise bf16 tensor_tensor tree
    on DVE (2x mode) + a short fp32 reduce tail.
  - The gene-table term is a segment-sum over the table, computed as a
    matmul: gsum[d,c] = sum_g gene[g,d] * hist[g,c], where hist is the
    per-chunk gene-id histogram (fp8, exact small counts) built on the
    host.  This replaces the row-gather entirely.
  - A tiny pair of fp32 matmuls projects t1 by [W_att1 | W_cls]/V, then
    the per-sample softmax runs with samples on partitions ([32, 8]).
"""

import functools
import os
import sys

import numpy as np

for _p in ("/opt/trn_rl_repo",):
    if _p not in sys.path and os.path.isdir(_p):
        sys.path.insert(0, _p)

import ml_dtypes  # noqa: E402

import concourse.bass as bass  # noqa: E402
import concourse.tile as tile  # noqa: E402
from concourse import bacc, mybir  # noqa: E402
from concourse.bass_utils import run_bass_kernel_spmd  # noqa: E402
from contextlib import ExitStack  # noqa: E402

F32 = mybir.dt.float32
BF16 = mybir.dt.bfloat16
FP8 = mybir.dt.float8e4
AF = mybir.ActivationFunctionType
ALU = mybir.AluOpType
AX = mybir.AxisListType

B, V, F, D, G, S = 2048, 256, 64, 256, 20000, 256
POS_SCALE = 1e-6
NCORES = 8
BC = B // NCORES          # 256 chunks per core
RC = BC * V               # 65536 rows per core
SC = S // NCORES          # 32 samples per core
K8 = B // S               # 8 chunks per sample
KIN = F + 1               # 65 = features + position row

ST_CH = 4                 # chunks per supertile (PSUM tile)
ST_ROWS = ST_CH * V       # 1024
NST = BC // ST_CH         # 64 supertiles
GRP_CH = 32               # chunks per DVE-tree group
GRP_ROWS = GRP_CH * V     # 8192
NGRP = BC // GRP_CH       # 8 groups
ST_PER_GRP = GRP_CH // ST_CH  # 8

GKT = (G + 127) // 128    # 157 gene k-tiles
GPAD = GKT * 128          # 20096
GCH = 16                  # gene k-tiles per DMA chunk
HPS = 6                   # hist k-tiles interleaved per supertile
HDELAY = 4                # supertiles before first hist matmul


def _emit(nc, tc, featT, geneT, histT, w65, bfeat, psc, batt1, watt2, bcls,
          out):
    ctx = ExitStack()
    with ctx:
        const = ctx.enter_context(tc.tile_pool(name="const", bufs=1))
        acc = ctx.enter_context(tc.tile_pool(name="acc", bufs=1))
        feat_p = ctx.enter_context(tc.tile_pool(name="feat", bufs=3))
        gene_p = ctx.enter_context(tc.tile_pool(name="gene", bufs=3))
        hist_p = ctx.enter_context(tc.tile_pool(name="hist", bufs=3))
        xg_p = ctx.enter_context(tc.tile_pool(name="xg", bufs=2))
        tree_p = ctx.enter_context(tc.tile_pool(name="tree", bufs=1))
        zpool = ctx.enter_context(tc.tile_pool(name="zp", bufs=3, space="PSUM"))
        gsum_p = ctx.enter_context(tc.tile_pool(name="gs", bufs=1, space="PSUM"))
        small = ctx.enter_context(tc.tile_pool(name="small", bufs=1))

        # ---- constants (first ft tile is prefetched between them) ----
        w65_t = const.tile([KIN, D], BF16)
        nc.sync.dma_start(w65_t[:, :], w65[:, :])
        ft0 = feat_p.tile([KIN, ST_ROWS], BF16, tag="ft", name="ft")
        nc.sync.dma_start(ft0[:, 0:512], featT[:, 0:512])
        nc.sync.dma_start(ft0[:, 512:1024], featT[:, 512:1024])
        bf_t = const.tile([128, 2], F32)
        nc.sync.dma_start(bf_t[:, 0:1], bfeat[0:128, :])
        nc.sync.dma_start(bf_t[:, 1:2], bfeat[128:256, :])
        psc_t0 = const.tile([128, KIN], F32)
        psc_t1 = const.tile([128, KIN], F32)
        nc.sync.dma_start(psc_t0[:, :], psc[0:128, :])
        nc.sync.dma_start(psc_t1[:, :], psc[128:256, :])
        batt1_t = const.tile([64, 1], F32)
        nc.sync.dma_start(batt1_t[:, :], batt1[:, :])
        watt2_t = const.tile([64, 1], F32)
        nc.sync.dma_start(watt2_t[:, :], watt2[:, :])
        bcls_t = const.tile([1, 1], F32)
        nc.sync.dma_start(bcls_t[:, :], bcls[:, :])

        # per-(D-half) accumulators
        t1 = [acc.tile([128, BC], F32, tag=f"t1_{h}", name=f"t1_{h}")
              for h in range(2)]
        # one PSUM bank holds both gsum halves side by side; a second holds
        # the projection outputs (h = [W_att1|W_cls]^T emb) and the scores
        gsum_b = gsum_p.tile([128, 2 * BC], F32, tag="gs", name="gsum_b")
        gsum = [gsum_b[:, h * BC:(h + 1) * BC] for h in range(2)]
        psHS = gsum_p.tile([128, 2 * BC], F32, tag="ph", name="psHS")
        xg = {}
        # tree scratch (shared across halves/groups; DVE is serial anyway)
        yA = tree_p.tile([128, GRP_ROWS // 2], BF16)
        yB = tree_p.tile([128, GRP_ROWS // 4], BF16)

        # DRAM views of gene table / hist: host supplies [128, kt, n]
        # partition-major layout (long contiguous per-partition lines)
        geneT_v = geneT.rearrange("p (t d) -> p t d", d=D)
        histT_v = histT.rearrange("p (t c) -> p t c", c=BC)

        # interleave schedule for the gene-histogram matmuls
        kt_next = [0]
        gene_cur = [None, None]  # (gene_tile, hist_tile)

        def emit_hist_upto(limit):
            while kt_next[0] < min(limit, GKT):
                kt = kt_next[0]
                if kt % GCH == 0:
                    cs = min(GCH, GKT - kt)
                    gt = gene_p.tile([128, GCH, D], BF16, tag="gt", name="gt")
                    ht = hist_p.tile([128, GCH, BC], FP8, tag="ht", name="ht")
                    k0 = kt
                    # split across several DMA queues (~20 GB/s per engine)
                    for j in range(0, cs, 4):
                        je = min(j + 4, cs)
                        nc.sync.dma_start(gt[:, j:je, :],
                                          geneT_v[:, k0 + j:k0 + je, :])
                    for j in range(0, cs, 8):
                        je = min(j + 8, cs)
                        nc.sync.dma_start(ht[:, j:je, :],
                                          histT_v[:, k0 + j:k0 + je, :])
                    gene_cur[0] = gt
                    gene_cur[1] = ht
                loc = kt % GCH
                for h in range(2):
                    nc.tensor.matmul(
                        gsum[h],
                        gene_cur[0][:, loc, h * 128:(h + 1) * 128],
                        gene_cur[1][:, loc, :],
                        start=(kt == 0),
                        stop=(kt == GKT - 1),
                    )
                kt_next[0] += 1

        def emit_group_tree(g):
            """V-sum tree for chunk-group g (consumes the current xg tiles)."""
            gs = slice(g * GRP_CH, (g + 1) * GRP_CH)
            for h in range(2):
                xv = xg[h].rearrange("p (c v) -> p c v", v=V)
                nc.vector.tensor_add(
                    yA[:, 0:4096].rearrange("p (c v) -> p c v", v=128),
                    xv[:, :, 0:128], xv[:, :, 128:256])
                a1 = yA[:, 0:4096].rearrange("p (c v) -> p c v", v=128)
                nc.vector.tensor_add(
                    yB[:, 0:2048].rearrange("p (c v) -> p c v", v=64),
                    a1[:, :, 0:64], a1[:, :, 64:128])
                a2 = yB[:, 0:2048].rearrange("p (c v) -> p c v", v=64)
                nc.vector.tensor_add(
                    yA[:, 0:1024].rearrange("p (c v) -> p c v", v=32),
                    a2[:, :, 0:32], a2[:, :, 32:64])
                a3 = yA[:, 0:1024].rearrange("p (c v) -> p c v", v=32)
                nc.vector.tensor_add(
                    yB[:, 0:512].rearrange("p (c v) -> p c v", v=16),
                    a3[:, :, 0:16], a3[:, :, 16:32])
                a4 = yB[:, 0:512].rearrange("p (c v) -> p c v", v=16)
                nc.vector.tensor_add(
                    yA[:, 0:256].rearrange("p (c v) -> p c v", v=8),
                    a4[:, :, 0:8], a4[:, :, 8:16])
                nc.vector.reduce_sum(
                    t1[h][:, gs],
                    yA[:, 0:256].rearrange("p (c v) -> p c v", v=8),
                    axis=AX.X,
                )

        def emit_group_proj(g):
            """Add the gene segment-sum and project chunk-group g.
            Deferred past the last hist matmul for early groups."""
            gs = slice(g * GRP_CH, (g + 1) * GRP_CH)
            for h in range(2):
                nc.vector.tensor_add(t1[h][:, gs], t1[h][:, gs],
                                     gsum[h][:, gs])
            hv = psHS[0:KIN, gs]
            nc.tensor.matmul(hv, psc_t0[:, :], t1[0][:, gs],
                             start=True, stop=False)
            nc.tensor.matmul(hv, psc_t1[:, :], t1[1][:, gs],
                             start=False, stop=True)

        # ---- main loop ----
        for g in range(NGRP):
            for h in range(2):
                xg[h] = xg_p.tile([128, GRP_ROWS], BF16, tag=f"xg_{h}",
                                  name=f"xg_{h}")
            for sl in range(ST_PER_GRP):
                s = g * ST_PER_GRP + sl
                if s == 0:
                    ft = ft0  # prefetched with the constants
                else:
                    ft = feat_p.tile([KIN, ST_ROWS], BF16, tag="ft", name="ft")
                    c0 = s * ST_ROWS
                    nc.sync.dma_start(ft[:, 0:512], featT[:, c0:c0 + 512])
                    nc.sync.dma_start(ft[:, 512:1024],
                                      featT[:, c0 + 512:c0 + 1024])
                for h in range(2):
                    zp = zpool.tile([128, ST_ROWS], F32, tag="ps", name="zp")
                    for q in range(ST_ROWS // 512):
                        nc.tensor.matmul(
                            zp[:, q * 512:(q + 1) * 512],
                            w65_t[:, h * 128:(h + 1) * 128],
                            ft[:, q * 512:(q + 1) * 512],
                            start=True,
                            stop=True,
                        )
                    nc.scalar.activation(
                        xg[h][:, sl * ST_ROWS:(sl + 1) * ST_ROWS],
                        zp[:, :], AF.Tanh, bias=bf_t[:, h:h + 1],
                    )
                emit_hist_upto(HPS * (s + 1 - HDELAY))

            emit_group_tree(g)
            if g == 3:
                emit_hist_upto(GKT)  # safety: finish any hist stragglers
                for gg in range(4):
                    emit_group_proj(gg)
            elif g > 3:
                emit_group_proj(g)

        # ---- attention scores + classifier from the projections ----
        u_t = small.tile([64, BC], F32)
        nc.scalar.activation(u_t[:, :], psHS[0:64, 0:BC], AF.Tanh,
                             bias=batt1_t[:, :])
        a_t = small.tile([1, BC], F32)
        # a = emb @ W_cls / V + b_cls  (adding b_cls here is fine: sum w = 1)
        nc.scalar.activation(a_t[:, :], psHS[64:65, 0:BC], AF.Identity,
                             bias=bcls_t[:, :])

        psS = psHS[0:1, BC:2 * BC]
        nc.tensor.matmul(psS, watt2_t[:, :], u_t[:, :],
                         start=True, stop=True)

        # ---- per-sample softmax over 8 chunks, all in [1, BC] layout.
        # Scores are bounded (|s| <= ||W_att2||_1 + |b|), so exp() without
        # the max-subtraction is safe in fp32 and softmax is shift-invariant.
        e_t = small.tile([1, BC], F32)
        nc.scalar.activation(e_t[:, :], psS, AF.Exp)
        e_v = e_t.rearrange("p (s k) -> p s k", k=K8)
        ssum = small.tile([1, SC], F32)
        nc.vector.reduce_sum(ssum[:, :], e_v, axis=AX.X)
        rec = small.tile([1, SC], F32)
        nc.vector.reciprocal(rec[:, :], ssum[:, :])
        wa = small.tile([1, BC], F32)
        nc.vector.tensor_mul(wa[:, :], e_t[:, :], a_t[:, :])
        was = small.tile([1, SC], F32)
        nc.vector.reduce_sum(was[:, :], wa.rearrange("p (s k) -> p s k", k=K8),
                             axis=AX.X)
        o_t = small.tile([1, SC], F32)
        nc.vector.tensor_mul(o_t[:, :], was[:, :], rec[:, :])
        nc.sync.dma_start(out.rearrange("s o -> o s"), o_t[:, :])


@functools.lru_cache(maxsize=1)
def _build():
    nc = bacc.Bacc(
        "TRN2",
        target_bir_lowering=False,
        debug=False,
        enable_asserts=False,
        num_devices=NCORES,
    )
    featT = nc.dram_tensor("featT", [KIN, RC], BF16, kind="ExternalInput")
    geneT = nc.dram_tensor("geneT", [128, GKT * D], BF16, kind="ExternalInput")
    histT = nc.dram_tensor("histT", [128, GKT * BC], FP8, kind="ExternalInput")
    w65 = nc.dram_tensor("w65", [KIN, D], BF16, kind="ExternalInput")
    bfeat = nc.dram_tensor("bfeat", [D, 1], F32, kind="ExternalInput")
    psc = nc.dram_tensor("psc", [D, KIN], F32, kind="ExternalInput")
    batt1 = nc.dram_tensor("batt1", [64, 1], F32, kind="ExternalInput")
    watt2 = nc.dram_tensor("watt2", [64, 1], F32, kind="ExternalInput")
    bcls = nc.dram_tensor("bcls", [1, 1], F32, kind="ExternalInput")
    out = nc.dram_tensor("out", [SC, 1], F32, kind="ExternalOutput")
    with tile.TileContext(nc) as tc:
        _emit(nc, tc, featT.ap(), geneT.ap(), histT.ap(), w65.ap(),
              bfeat.ap(), psc.ap(), batt1.ap(), watt2.ap(), bcls.ap(),
              out.ap())
    nc.compile()
    return nc


def _prep_inputs(features, positions, gene_ids, mask, original_sample_indices,
                 W_feat, b_feat, gene_table, w_pos,
                 W_att1, b_att1, W_att2, b_att2, W_cls, b_cls):
    features = np.asarray(features, np.float32)
    positions = np.asarray(positions)
    gene_ids = np.asarray(gene_ids)
    BF = ml_dtypes.bfloat16
    F8 = ml_dtypes.float8_e4m3fn

    featT_full = np.empty((KIN, B * V), BF)
    featT_full[:F] = features.reshape(B * V, F).T.astype(BF)
    featT_full[F] = (positions.reshape(-1).astype(np.float32)
                     * POS_SCALE).astype(BF)

    gene_bf = np.zeros((GPAD, D), BF)
    gene_bf[:G] = np.asarray(gene_table, np.float32).astype(BF)
    # partition-major layout: [128, GKT, D] -> [128, GKT*D]
    gene_pm = np.ascontiguousarray(
        gene_bf.reshape(GKT, 128, D).transpose(1, 0, 2)).reshape(128, GKT * D)

    ids = gene_ids.reshape(B, V).astype(np.int64)
    chunk_local = (np.arange(B)[:, None] % BC).repeat(V, axis=1)

    w65v = np.concatenate(
        [np.asarray(W_feat, np.float32),
         np.asarray(w_pos, np.float32)[None, :]], axis=0).astype(BF)
    pscv = np.ascontiguousarray(
        np.concatenate([np.asarray(W_att1, np.float32),
                        np.asarray(W_cls, np.float32)], axis=1) / V)
    bfeatv = np.ascontiguousarray(np.asarray(b_feat, np.float32)[:, None])
    batt1v = np.ascontiguousarray(np.asarray(b_att1, np.float32)[:, None])
    watt2v = np.ascontiguousarray(np.asarray(W_att2, np.float32))
    bclsv = np.asarray(b_cls, np.float32).reshape(1, 1)

    in_maps = []
    for c in range(NCORES):
        ids_c = ids[c * BC:(c + 1) * BC].reshape(-1)
        loc_c = chunk_local[c * BC:(c + 1) * BC].reshape(-1)
        hist = np.bincount(ids_c * BC + loc_c,
                           minlength=GPAD * BC).reshape(GPAD, BC)
        hist_pm = np.ascontiguousarray(
            hist.reshape(GKT, 128, BC).transpose(1, 0, 2)
        ).reshape(128, GKT * BC)
        in_maps.append({
            "featT": np.ascontiguousarray(featT_full[:, c * RC:(c + 1) * RC]),
            "geneT": gene_pm,
            "histT": hist_pm.astype(F8),
            "w65": w65v,
            "bfeat": bfeatv,
            "psc": pscv,
            "batt1": batt1v,
            "watt2": watt2v,
            "bcls": bclsv,
        })
    return in_maps


def _run(inputs, trace=False, **kw):
    nc = _build()
    in_maps = _prep_inputs(**inputs)
    res = run_bass_kernel_spmd(
        nc, in_maps, core_ids=list(range(NCORES)), trace=trace, **kw)
    outv = np.concatenate(
        [np.asarray(res.results[c]["out"], np.float32) for c in range(NCORES)],
        axis=0)
    return outv, res


def _numpy_fallback(features, positions, gene_ids, mask,
                    original_sample_indices, W_feat, b_feat, gene_table,
                    w_pos, W_att1, b_att1, W_att2, b_att2, W_cls, b_cls):
    features = np.asarray(features, np.float32)
    mask_f = np.asarray(mask, np.float32)
    pos = np.asarray(positions).astype(np.float32) * POS_SCALE
    x = np.tanh(features @ np.asarray(W_feat, np.float32)
                + np.asarray(b_feat, np.float32)
                + pos[..., None] * np.asarray(w_pos, np.float32))
    x = x + np.asarray(gene_table, np.float32)[np.asarray(gene_ids)]
    denom = np.maximum(mask_f.sum(-1, keepdims=True), 1.0)
    emb = (x * mask_f[..., None]).sum(axis=1) / denom
    scores = (np.tanh(emb @ np.asarray(W_att1, np.float32)
                      + np.asarray(b_att1, np.float32))
              @ np.asarray(W_att2, np.float32)
              + np.asarray(b_att2, np.float32))[:, 0]
    seg = np.asarray(original_sample_indices).astype(np.int64)
    smax = np.full(S, -np.inf, np.float32)
    np.maximum.at(smax, seg, scores)
    e = np.exp(scores - smax[seg])
    ssum = np.zeros(S, np.float32)
    np.add.at(ssum, seg, e)
    w = e / ssum[seg]
    agg = np.zeros((S, D), np.float32)
    np.add.at(agg, seg, emb * w[:, None])
    return agg @ np.asarray(W_cls, np.float32) + np.asarray(b_cls, np.float32)


def kernel(**inputs):
    mask = np.asarray(inputs["mask"])
    seg = np.asarray(inputs["original_sample_indices"]).astype(np.int64)
    expected_seg = np.arange(B) // K8
    if not mask.all() or not np.array_equal(seg, expected_seg):
        return _numpy_fallback(**inputs)
    outv, _ = _run(inputs)
    return outv


# revision 26
# speedup vs baseline: 1.1489x; 1.1489x over previous
"""Trainium2 Bass kernel for the ChunkedSIEVE model (segment_reduce).

Math (see reference):
  x[b,v,:]  = tanh(feat[b,v,:] @ W_feat + b_feat + pos[b,v]*1e-6 * w_pos)
              + gene_table[gene_ids[b,v]]
  emb[b]    = mean_v x[b,v,:]                      (mask is all ones)
  scores[b] = tanh(emb @ W_att1 + b_att1) @ W_att2 (+ b_att2, cancels in softmax)
  per-sample (8 contiguous chunks) softmax over scores -> w
  out[s]    = sum_b w[b] * (emb[b] @ W_cls) + b_cls

Strategy: data-parallel over chunks, 256 chunks (32 samples) per core.
Everything stays in [D x chunk] layout:
  - PE computes z = [W_feat; w_pos]^T @ featT in bf16 (K=65: 64 features +
    the scaled-position row appended on the host).
  - ACT applies tanh with the per-partition b_feat bias straight out of
    PSUM, writing bf16 tiles.
  - The V-sum of the tanh term runs as a pairwise bf16 tensor_tensor tree
    on DVE (2x mode) + a short fp32 reduce tail.
  - The gene-table term is a segment-sum over the table, computed as a
    matmul: gsum[d,c] = sum_g gene[g,d] * hist[g,c], where hist is the
    per-chunk gene-id histogram (fp8, exact small counts) built on the
    host.  This replaces the row-gather entirely.
  - A tiny pair of fp32 matmuls projects t1 by [W_att1 | W_cls]/V, then
    the per-sample softmax runs with samples on partitions ([32, 8]).
"""

import functools
import os
import sys

import numpy as np

for _p in ("/opt/trn_rl_repo",):
    if _p not in sys.path and os.path.isdir(_p):
        sys.path.insert(0, _p)

import ml_dtypes  # noqa: E402

import concourse.bass as bass  # noqa: E402
import concourse.tile as tile  # noqa: E402
from concourse import bacc, mybir  # noqa: E402
from concourse.bass_utils import run_bass_kernel_spmd  # noqa: E402
from contextlib import ExitStack  # noqa: E402

F32 = mybir.dt.float32
BF16 = mybir.dt.bfloat16
FP8 = mybir.dt.float8e4
AF = mybir.ActivationFunctionType
ALU = mybir.AluOpType
AX = mybir.AxisListType

B, V, F, D, G, S = 2048, 256, 64, 256, 20000, 256
POS_SCALE = 1e-6
NCORES = 8
BC = B // NCORES          # 256 chunks per core
RC = BC * V               # 65536 rows per core
SC = S // NCORES          # 32 samples per core
K8 = B // S               # 8 chunks per sample
KIN = F + 1               # 65 = features + position row

ST_CH = 4                 # chunks per supertile (PSUM tile)
ST_ROWS = ST_CH * V       # 1024
NST = BC // ST_CH         # 64 supertiles
GRP_CH = 32               # chunks per DVE-tree group
GRP_ROWS = GRP_CH * V     # 8192
NGRP = BC // GRP_CH       # 8 groups
ST_PER_GRP = GRP_CH // ST_CH  # 8

FT_ST = 4                 # supertiles per feature DMA tile
FT_ROWS = FT_ST * ST_ROWS  # 4096
GKT = (G + 127) // 128    # 157 gene k-tiles
GPAD = GKT * 128          # 20096
GCH = 8                   # gene k-tiles per DMA chunk (one trigger each)
HPS = 5                   # hist k-tiles interleaved per supertile
HDELAY = 4                # supertiles before first hist matmul
GENE_SCALE = 64.0         # fp8 range scale for the gene table


def _emit(nc, tc, featT, geneT, histT, w65, bfeat, psc, batt1, watt2, bcls,
          out):
    ctx = ExitStack()
    with ctx:
        const = ctx.enter_context(tc.tile_pool(name="const", bufs=1))
        acc = ctx.enter_context(tc.tile_pool(name="acc", bufs=1))
        feat_p = ctx.enter_context(tc.tile_pool(name="feat", bufs=6))
        gene_p = ctx.enter_context(tc.tile_pool(name="gene", bufs=6))
        hist_p = ctx.enter_context(tc.tile_pool(name="hist", bufs=6))
        xg_p = ctx.enter_context(tc.tile_pool(name="xg", bufs=2))
        tree_p = ctx.enter_context(tc.tile_pool(name="tree", bufs=1))
        zpool = ctx.enter_context(tc.tile_pool(name="zp", bufs=3, space="PSUM"))
        gsum_p = ctx.enter_context(tc.tile_pool(name="gs", bufs=1, space="PSUM"))
        small = ctx.enter_context(tc.tile_pool(name="small", bufs=1))

        # ---- constants (first ft tile is prefetched between them) ----
        w65_t = const.tile([KIN, D], BF16)
        nc.sync.dma_start(w65_t[:, :], w65[:, :])
        ft0 = feat_p.tile([KIN, FT_ROWS], BF16, tag="ft", name="ft")
        for j in range(FT_ST):
            nc.sync.dma_start(ft0[:, j * ST_ROWS:(j + 1) * ST_ROWS],
                              featT[:, j * ST_ROWS:(j + 1) * ST_ROWS])
        bf_t = const.tile([128, 2], F32)
        nc.sync.dma_start(bf_t[:, 0:1], bfeat[0:128, :])
        nc.sync.dma_start(bf_t[:, 1:2], bfeat[128:256, :])
        psc_t0 = const.tile([128, KIN], F32)
        psc_t1 = const.tile([128, KIN], F32)
        nc.sync.dma_start(psc_t0[:, :], psc[0:128, :])
        nc.sync.dma_start(psc_t1[:, :], psc[128:256, :])
        batt1_t = const.tile([64, 1], F32)
        nc.sync.dma_start(batt1_t[:, :], batt1[:, :])
        watt2_t = const.tile([64, 1], F32)
        nc.sync.dma_start(watt2_t[:, :], watt2[:, :])
        bcls_t = const.tile([1, 1], F32)
        nc.sync.dma_start(bcls_t[:, :], bcls[:, :])

        # per-(D-half) accumulators
        t1 = [acc.tile([128, BC], F32, tag=f"t1_{h}", name=f"t1_{h}")
              for h in range(2)]
        # one PSUM bank holds both gsum halves side by side; a second holds
        # the projection outputs (h = [W_att1|W_cls]^T emb) and the scores
        gsum_b = gsum_p.tile([128, 2 * BC], F32, tag="gs", name="gsum_b")
        gsum = [gsum_b[:, h * BC:(h + 1) * BC] for h in range(2)]
        psHS = gsum_p.tile([128, 2 * BC], F32, tag="ph", name="psHS")
        xg = {}
        # tree scratch (shared across halves/groups; DVE is serial anyway)
        yA = tree_p.tile([128, GRP_ROWS // 2], BF16)
        yB = tree_p.tile([128, GRP_ROWS // 4], BF16)

        # DRAM views of gene table / hist: host supplies [128, kt, n]
        # partition-major layout (long contiguous per-partition lines)
        geneT_v = geneT.rearrange("p (t d) -> p t d", d=D)
        histT_v = histT.rearrange("p (t c) -> p t c", c=BC)

        # interleave schedule for the gene-histogram matmuls
        kt_next = [0]
        gene_cur = [None, None]  # (gene_tile, hist_tile)

        def emit_hist_upto(limit):
            while kt_next[0] < min(limit, GKT):
                kt = kt_next[0]
                if kt % GCH == 0:
                    cs = min(GCH, GKT - kt)
                    gt = gene_p.tile([128, GCH, D], FP8, tag="gt", name="gt")
                    ht = hist_p.tile([128, GCH, BC], FP8, tag="ht", name="ht")
                    nc.sync.dma_start(gt[:, 0:cs, :],
                                      geneT_v[:, kt:kt + cs, :])
                    nc.sync.dma_start(ht[:, 0:cs, :],
                                      histT_v[:, kt:kt + cs, :])
                    gene_cur[0] = gt
                    gene_cur[1] = ht
                loc = kt % GCH
                for h in range(2):
                    nc.tensor.matmul(
                        gsum[h],
                        gene_cur[0][:, loc, h * 128:(h + 1) * 128],
                        gene_cur[1][:, loc, :],
                        start=(kt == 0),
                        stop=(kt == GKT - 1),
                    )
                kt_next[0] += 1

        def emit_group_tree(g):
            """V-sum tree for chunk-group g (consumes the current xg tiles)."""
            gs = slice(g * GRP_CH, (g + 1) * GRP_CH)
            for h in range(2):
                xv = xg[h].rearrange("p (c v) -> p c v", v=V)
                nc.vector.tensor_add(
                    yA[:, 0:4096].rearrange("p (c v) -> p c v", v=128),
                    xv[:, :, 0:128], xv[:, :, 128:256])
                a1 = yA[:, 0:4096].rearrange("p (c v) -> p c v", v=128)
                nc.vector.tensor_add(
                    yB[:, 0:2048].rearrange("p (c v) -> p c v", v=64),
                    a1[:, :, 0:64], a1[:, :, 64:128])
                a2 = yB[:, 0:2048].rearrange("p (c v) -> p c v", v=64)
                nc.vector.tensor_add(
                    yA[:, 0:1024].rearrange("p (c v) -> p c v", v=32),
                    a2[:, :, 0:32], a2[:, :, 32:64])
                a3 = yA[:, 0:1024].rearrange("p (c v) -> p c v", v=32)
                nc.vector.tensor_add(
                    yB[:, 0:512].rearrange("p (c v) -> p c v", v=16),
                    a3[:, :, 0:16], a3[:, :, 16:32])
                a4 = yB[:, 0:512].rearrange("p (c v) -> p c v", v=16)
                nc.vector.tensor_add(
                    yA[:, 0:256].rearrange("p (c v) -> p c v", v=8),
                    a4[:, :, 0:8], a4[:, :, 8:16])
                nc.vector.reduce_sum(
                    t1[h][:, gs],
                    yA[:, 0:256].rearrange("p (c v) -> p c v", v=8),
                    axis=AX.X,
                )

        def emit_group_proj(g):
            """Add the (rescaled) gene segment-sum and project chunk-group g.
            Deferred past the last hist matmul for early groups."""
            gs = slice(g * GRP_CH, (g + 1) * GRP_CH)
            for h in range(2):
                nc.vector.scalar_tensor_tensor(
                    t1[h][:, gs], gsum[h][:, gs], 1.0 / GENE_SCALE,
                    t1[h][:, gs], op0=ALU.mult, op1=ALU.add)
            hv = psHS[0:KIN, gs]
            nc.tensor.matmul(hv, psc_t0[:, :], t1[0][:, gs],
                             start=True, stop=False)
            nc.tensor.matmul(hv, psc_t1[:, :], t1[1][:, gs],
                             start=False, stop=True)

        # ---- main loop ----
        for g in range(NGRP):
            for h in range(2):
                xg[h] = xg_p.tile([128, GRP_ROWS], BF16, tag=f"xg_{h}",
                                  name=f"xg_{h}")
            for sl in range(ST_PER_GRP):
                s = g * ST_PER_GRP + sl
                if s == 0:
                    ft = ft0  # prefetched with the constants
                elif s % FT_ST == 0:
                    ft = feat_p.tile([KIN, FT_ROWS], BF16, tag="ft", name="ft")
                    c0 = s * ST_ROWS
                    nc.sync.dma_start(ft[:, :], featT[:, c0:c0 + FT_ROWS])
                off = (s % FT_ST) * ST_ROWS
                for h in range(2):
                    zp = zpool.tile([128, ST_ROWS], F32, tag="ps", name="zp")
                    for q in range(ST_ROWS // 512):
                        nc.tensor.matmul(
                            zp[:, q * 512:(q + 1) * 512],
                            w65_t[:, h * 128:(h + 1) * 128],
                            ft[:, off + q * 512:off + (q + 1) * 512],
                            start=True,
                            stop=True,
                        )
                    nc.scalar.activation(
                        xg[h][:, sl * ST_ROWS:(sl + 1) * ST_ROWS],
                        zp[:, :], AF.Tanh, bias=bf_t[:, h:h + 1],
                    )
                emit_hist_upto(HPS * (s + 1 - HDELAY))

            emit_group_tree(g)
            if g == 5:
                emit_hist_upto(GKT)  # safety: finish any hist stragglers
                for gg in range(6):
                    emit_group_proj(gg)
            elif g > 5:
                emit_group_proj(g)

        # ---- attention scores + classifier from the projections ----
        u_t = small.tile([64, BC], F32)
        nc.scalar.activation(u_t[:, :], psHS[0:64, 0:BC], AF.Tanh,
                             bias=batt1_t[:, :])
        a_t = small.tile([1, BC], F32)
        # a = emb @ W_cls / V + b_cls  (adding b_cls here is fine: sum w = 1)
        nc.scalar.activation(a_t[:, :], psHS[64:65, 0:BC], AF.Identity,
                             bias=bcls_t[:, :])

        psS = psHS[0:1, BC:2 * BC]
        nc.tensor.matmul(psS, watt2_t[:, :], u_t[:, :],
                         start=True, stop=True)

        # ---- per-sample softmax over 8 chunks, all in [1, BC] layout.
        # Scores are bounded (|s| <= ||W_att2||_1 + |b|), so exp() without
        # the max-subtraction is safe in fp32 and softmax is shift-invariant.
        e_t = small.tile([1, BC], F32)
        nc.scalar.activation(e_t[:, :], psS, AF.Exp)
        e_v = e_t.rearrange("p (s k) -> p s k", k=K8)
        ssum = small.tile([1, SC], F32)
        nc.vector.reduce_sum(ssum[:, :], e_v, axis=AX.X)
        rec = small.tile([1, SC], F32)
        nc.vector.reciprocal(rec[:, :], ssum[:, :])
        wa = small.tile([1, BC], F32)
        nc.vector.tensor_mul(wa[:, :], e_t[:, :], a_t[:, :])
        was = small.tile([1, SC], F32)
        nc.vector.reduce_sum(was[:, :], wa.rearrange("p (s k) -> p s k", k=K8),
                             axis=AX.X)
        o_t = small.tile([1, SC], F32)
        nc.vector.tensor_mul(o_t[:, :], was[:, :], rec[:, :])
        nc.sync.dma_start(out.rearrange("s o -> o s"), o_t[:, :])


@functools.lru_cache(maxsize=1)
def _build():
    nc = bacc.Bacc(
        "TRN2",
        target_bir_lowering=False,
        debug=False,
        enable_asserts=False,
        num_devices=NCORES,
    )
    featT = nc.dram_tensor("featT", [KIN, RC], BF16, kind="ExternalInput")
    geneT = nc.dram_tensor("geneT", [128, GKT * D], FP8, kind="ExternalInput")
    histT = nc.dram_tensor("histT", [128, GKT * BC], FP8, kind="ExternalInput")
    w65 = nc.dram_tensor("w65", [KIN, D], BF16, kind="ExternalInput")
    bfeat = nc.dram_tensor("bfeat", [D, 1], F32, kind="ExternalInput")
    psc = nc.dram_tensor("psc", [D, KIN], F32, kind="ExternalInput")
    batt1 = nc.dram_tensor("batt1", [64, 1], F32, kind="ExternalInput")
    watt2 = nc.dram_tensor("watt2", [64, 1], F32, kind="ExternalInput")
    bcls = nc.dram_tensor("bcls", [1, 1], F32, kind="ExternalInput")
    out = nc.dram_tensor("out", [SC, 1], F32, kind="ExternalOutput")
    with tile.TileContext(nc) as tc:
        _emit(nc, tc, featT.ap(), geneT.ap(), histT.ap(), w65.ap(),
              bfeat.ap(), psc.ap(), batt1.ap(), watt2.ap(), bcls.ap(),
              out.ap())
    nc.compile()
    return nc


def _prep_inputs(features, positions, gene_ids, mask, original_sample_indices,
                 W_feat, b_feat, gene_table, w_pos,
                 W_att1, b_att1, W_att2, b_att2, W_cls, b_cls):
    features = np.asarray(features, np.float32)
    positions = np.asarray(positions)
    gene_ids = np.asarray(gene_ids)
    BF = ml_dtypes.bfloat16
    F8 = ml_dtypes.float8_e4m3fn

    featT_full = np.empty((KIN, B * V), BF)
    featT_full[:F] = features.reshape(B * V, F).T.astype(BF)
    featT_full[F] = (positions.reshape(-1).astype(np.float32)
                     * POS_SCALE).astype(BF)

    gene_f8 = np.zeros((GPAD, D), F8)
    gene_f8[:G] = (np.asarray(gene_table, np.float32) * GENE_SCALE).astype(F8)
    # partition-major layout: [128, GKT, D] -> [128, GKT*D]
    gene_pm = np.ascontiguousarray(
        gene_f8.reshape(GKT, 128, D).transpose(1, 0, 2)).reshape(128, GKT * D)

    ids = gene_ids.reshape(B, V).astype(np.int64)
    chunk_local = (np.arange(B)[:, None] % BC).repeat(V, axis=1)

    w65v = np.concatenate(
        [np.asarray(W_feat, np.float32),
         np.asarray(w_pos, np.float32)[None, :]], axis=0).astype(BF)
    pscv = np.ascontiguousarray(
        np.concatenate([np.asarray(W_att1, np.float32),
                        np.asarray(W_cls, np.float32)], axis=1) / V)
    bfeatv = np.ascontiguousarray(np.asarray(b_feat, np.float32)[:, None])
    batt1v = np.ascontiguousarray(np.asarray(b_att1, np.float32)[:, None])
    watt2v = np.ascontiguousarray(np.asarray(W_att2, np.float32))
    bclsv = np.asarray(b_cls, np.float32).reshape(1, 1)

    in_maps = []
    for c in range(NCORES):
        ids_c = ids[c * BC:(c + 1) * BC].reshape(-1)
        loc_c = chunk_local[c * BC:(c + 1) * BC].reshape(-1)
        hist = np.bincount(ids_c * BC + loc_c,
                           minlength=GPAD * BC).reshape(GPAD, BC)
        hist_pm = np.ascontiguousarray(
            hist.reshape(GKT, 128, BC).transpose(1, 0, 2)
        ).reshape(128, GKT * BC)
        in_maps.append({
            "featT": np.ascontiguousarray(featT_full[:, c * RC:(c + 1) * RC]),
            "geneT": gene_pm,
            "histT": hist_pm.astype(F8),
            "w65": w65v,
            "bfeat": bfeatv,
            "psc": pscv,
            "batt1": batt1v,
            "watt2": watt2v,
            "bcls": bclsv,
        })
    return in_maps


def _run(inputs, trace=False, **kw):
    nc = _build()
    in_maps = _prep_inputs(**inputs)
    res = run_bass_kernel_spmd(
        nc, in_maps, core_ids=list(range(NCORES)), trace=trace, **kw)
    outv = np.concatenate(
        [np.asarray(res.results[c]["out"], np.float32) for c in range(NCORES)],
        axis=0)
    return outv, res


def _numpy_fallback(features, positions, gene_ids, mask,
                    original_sample_indices, W_feat, b_feat, gene_table,
                    w_pos, W_att1, b_att1, W_att2, b_att2, W_cls, b_cls):
    features = np.asarray(features, np.float32)
    mask_f = np.asarray(mask, np.float32)
    pos = np.asarray(positions).astype(np.float32) * POS_SCALE
    x = np.tanh(features @ np.asarray(W_feat, np.float32)
                + np.asarray(b_feat, np.float32)
                + pos[..., None] * np.asarray(w_pos, np.float32))
    x = x + np.asarray(gene_table, np.float32)[np.asarray(gene_ids)]
    denom = np.maximum(mask_f.sum(-1, keepdims=True), 1.0)
    emb = (x * mask_f[..., None]).sum(axis=1) / denom
    scores = (np.tanh(emb @ np.asarray(W_att1, np.float32)
                      + np.asarray(b_att1, np.float32))
              @ np.asarray(W_att2, np.float32)
              + np.asarray(b_att2, np.float32))[:, 0]
    seg = np.asarray(original_sample_indices).astype(np.int64)
    smax = np.full(S, -np.inf, np.float32)
    np.maximum.at(smax, seg, scores)
    e = np.exp(scores - smax[seg])
    ssum = np.zeros(S, np.float32)
    np.add.at(ssum, seg, e)
    w = e / ssum[seg]
    agg = np.zeros((S, D), np.float32)
    np.add.at(agg, seg, emb * w[:, None])
    return agg @ np.asarray(W_cls, np.float32) + np.asarray(b_cls, np.float32)


def kernel(**inputs):
    mask = np.asarray(inputs["mask"])
    seg = np.asarray(inputs["original_sample_indices"]).astype(np.int64)
    expected_seg = np.arange(B) // K8
    if not mask.all() or not np.array_equal(seg, expected_seg):
        return _numpy_fallback(**inputs)
    outv, _ = _run(inputs)
    return outv
